# revision 23
# baseline (speedup 1.0000x reference)
"""Trainium2 Bass kernel for nn_MultiHeadMapAttentionV2.

Math restructuring (v2 — host-side query path extended through softmax):
  - The 5-stage 1x1 conv chain is affine; only the mean token of its output
    feeds the (single) query: queries = W_tot @ mean_spatial(loss_map) + const.
  - pos_kv is pre-added into the feature-map tokens on host (xhat), so the
    device V projection is a single wv @ xhat matmul chain; the mean token
    collapses into the softmax-weight vector (W~_t = p_t + p_0/196) plus a
    constant column vc0 = wv @ (pos_0 - mean(pos_1:)) scaled by p_0.
  - Scores (q-side) are 1.5% of total FLOPs and depend on host-known q, so
    the softmax weights W~ are computed on host and shipped per group
    (197 bf16 cols), removing the on-device scores matmuls + softmax chain.
  - All device matmul operands are bf16 (fp32r streams ~2 cyc/row on HW;
    bf16 streams 1 cyc/row and halves DMA bytes).

Device structure (per core, Bc = 32 batches, G = 16 groups of 2):
  Channels-on-partitions, tokens-on-free. Per group ONE packed bf16 DMA
  ([128, 3336]: 8 fm k-chunks | wc block). Per m-tile: 8 bf16 matmuls
  (wv stationary) -> ps_v [128, 392]; Act copies cast to v_sb bf16
  (third col = vc0, written once per rotation slot); selector matmul
  broadcasts wc [8, 197] -> ps_w [128, 197]; one DVE scalar_tensor_tensor
  per (m, j) does the attention-weighted sum straight into outcat.
  Tail: wo projection on PE, LN stats via ones-matmul, PE transpose to
  [Bc, 1024], normalize.
"""

import numpy as np

P = 128
C = 1024
S = 14
SP = S * S          # 196 spatial tokens
NT = SP + 1         # 197 tokens
H = 8
DK = 64
NCORES = 8
B_FULL = 256
EPS = 1e-5

FM_COLS = 8 * 2 * SP            # 3136 (8 k-chunks x 2 batches x 196)
WC_OFF = FM_COLS                # 3136
WC_COLS = 4 * 2 * NT            # 1576: per m-tile [2*NT] pre-broadcast weights
GD_COLS = WC_OFF + WC_COLS      # 4712


# ---------------------------------------------------------------- host prep

def _host_prep(inputs):
    f = {k: np.ascontiguousarray(np.asarray(v, dtype=np.float32)) for k, v in inputs.items()}
    w1, w2, w3, w4, w5 = f['w1'], f['w2'], f['w3'], f['w4'], f['w5']
    b1, b2, b3, b4, b5 = f['b1'], f['b2'], f['b3'], f['b4'], f['b5']
    B = f['feature_map'].shape[0]

    Wt = w5 @ w4 @ w3 @ w2 @ w1                                   # (1024, 8)
    bt = w5 @ (w4 @ (w3 @ (w2 @ b1 + b2) + b3) + b4) + b5         # (1024,)
    lmean = f['loss_map'].reshape(B, 8, SP).mean(-1)              # (B, 8)
    queries = lmean @ Wt.T + bt + f['pos_q'][0]                   # (B, 1024)
    q = (queries @ f['wq'].T + f['bq']) / np.float32(np.sqrt(DK)) # (B, 512)
    qr_ = q.reshape(B, H, DK)
    wk_r = f['wk'].reshape(H, DK, C)
    Qt = np.einsum('hdc,bhd->bch', wk_r, qr_)                     # (B, 1024, 8)

    pos = f['pos_kv']                                             # (197, 1024)
    c0 = pos[0] - pos[1:].mean(0)                                 # (1024,)
    posT = np.ascontiguousarray(pos[1:].T)                        # (1024, 196)

    fm = f['feature_map'].reshape(B, C, SP)                       # (B, 1024, 196)
    xhat = fm + posT[None]                                        # (B, 1024, 196)

    # ---- host scores + softmax (q-side: ~1.5% of FLOPs)
    # s~_t = Qt . xhat_t ; bk shifts all scores equally -> softmax-invariant
    s_all = np.matmul(Qt.transpose(0, 2, 1), xhat)                # (B, 8, 196)
    sc0 = np.einsum('bch,c->bh', Qt, c0)                          # (B, 8)
    smean = s_all.mean(-1) + sc0                                  # (B, 8) mean-token score
    M = np.maximum(s_all.max(-1), smean)
    p_sp = np.exp(s_all - M[..., None])                           # (B, 8, 196)
    p_m = np.exp(smean - M)                                       # (B, 8)
    den = p_sp.sum(-1) + p_m
    Wsp = (p_sp + p_m[..., None] / SP) / den[..., None]           # (B, 8, 196)
    w0 = p_m / den                                                # (B, 8)

    # ---- shared (batch-independent) device arrays
    import ml_dtypes
    bf16 = ml_dtypes.bfloat16
    wv = f['wv']                                                  # (512, 1024)
    # wvt[p, (m*8+k)*128 + j] = wv[128m+j, 128k+p]
    wvt = np.ascontiguousarray(
        wv.reshape(4, P, 8, P).transpose(3, 0, 2, 1).reshape(P, 4096)).astype(bf16)
    wo = f['wo']                                                  # (1024, 512)
    # wot[p, (m8*4+k4)*128 + j] = wo[128*m8+j, 128*k4+p]
    wot = np.ascontiguousarray(
        wo.reshape(8, P, 4, P).transpose(3, 0, 2, 1).reshape(P, 4096)).astype(bf16)
    # vc0 column per m-tile: vc0a[p, m] = (wv @ c0)[128m + p]
    vc0 = wv @ c0                                                 # (512,)
    vc0a = np.ascontiguousarray(vc0.reshape(4, P).T)              # (128, 4)
    # gdiag[p, m8*128 + c] = ln_g[m8*128 + c] if p == c else 0 — the tail
    # transpose matmul res.T @ gdiag folds the LN gain for free
    gdiag = np.zeros((P, C), np.float32)
    for m8 in range(8):
        gdiag[np.arange(P), m8 * P + np.arange(P)] = f['ln_g'][m8 * P:(m8 + 1) * P]

    shared = {'wvt': wvt, 'wot': wot, 'vc0': vc0a, 'gdiag': gdiag}

    # out bias: v bias bv contributes wo @ bv (sum of p = 1)
    qpb = queries + f['bo'] + f['wo'] @ f['bv']                   # (B, 1024)

    def per_core(bs, be):
        Bc = be - bs
        G = Bc // 2
        gd = np.zeros((G, P, GD_COLS), bf16)
        # fm block: [p, (k*2 + j)*196 + t] = xhat[bs + 2g+j, 128k+p, t]
        gd[:, :, 0:FM_COLS] = (
            xhat[bs:be].reshape(G, 2, 8, P, SP).transpose(0, 3, 2, 1, 4)
            .reshape(G, P, FM_COLS).astype(bf16))
        # pre-broadcast weight block: wcb[g, p, m, j*197 + t] =
        # W~[batch 2g+j, head 2m + p//64, t]; col t=196 is p0 (vc0 weight).
        Wfull = np.concatenate([Wsp, w0[..., None]], -1)          # (B, 8, 197)
        heads = 2 * np.arange(4)[:, None] + (np.arange(P) // 64)[None, :]  # (4,128)
        b0 = Wfull[bs:be:2][:, heads, :]                          # (G, 4, 128, 197)
        b1 = Wfull[bs + 1:be:2][:, heads, :]
        wcb = np.stack([b0, b1], axis=3)                          # (G, 4, 128, 2, 197)
        wcb = wcb.transpose(0, 2, 1, 3, 4).reshape(G, P, WC_COLS)
        gd[:, :, WC_OFF:WC_OFF + WC_COLS] = wcb.astype(bf16)
        # qT[p, m*Bc + b] = qpb[bs + b, 128m + p]
        qT = np.ascontiguousarray(
            qpb[bs:be].T.reshape(8, P, Bc).transpose(1, 0, 2).reshape(P, 8 * Bc))
        grep = np.ascontiguousarray(np.broadcast_to(f['ln_g'], (Bc // 2, C)))
        brep = np.ascontiguousarray(np.broadcast_to(f['ln_b'], (Bc // 2, C)))
        return {'gdata': np.ascontiguousarray(gd), 'qT': qT,
                'grep': grep, 'brep': brep, **shared}

    return per_core


# ---------------------------------------------------------------- bass build

def build_bass(G=16, debug=False):
    import concourse.bacc as bacc
    import concourse.mybir as mybir
    import concourse.tile as tile

    f32 = mybir.dt.float32
    bf = mybir.dt.bfloat16
    Ax = mybir.AxisListType
    Op = mybir.AluOpType
    AF = mybir.ActivationFunctionType

    Bc = 2 * G
    nc = bacc.Bacc(trn_type="TRN2", name="mhma_v2")

    gd_d = nc.dram_tensor('gdata', (G, P, GD_COLS), bf, kind="ExternalInput")
    wvt_d = nc.dram_tensor('wvt', (P, 4096), bf, kind="ExternalInput")
    wot_d = nc.dram_tensor('wot', (P, 4096), bf, kind="ExternalInput")
    vc0_d = nc.dram_tensor('vc0', (P, 4), f32, kind="ExternalInput")
    qT_d = nc.dram_tensor('qT', (P, 8 * Bc), f32, kind="ExternalInput")
    grep_d = nc.dram_tensor('grep', (Bc // 2, C), f32, kind="ExternalInput")
    brep_d = nc.dram_tensor('brep', (Bc // 2, C), f32, kind="ExternalInput")
    gdiag_d = nc.dram_tensor('gdiag', (P, C), f32, kind="ExternalInput")
    out_d = nc.dram_tensor('out', (Bc, C), f32, kind="ExternalOutput")
    if debug:
        dbg_v_d = nc.dram_tensor('dbg_v', (P, 2 * NT), f32, kind="ExternalOutput")
        dbg_oc_d = nc.dram_tensor('dbg_oc', (P, 4 * Bc), f32, kind="ExternalOutput")

    with tile.TileContext(nc) as tc:
        with tc.tile_pool(name="const", bufs=1) as cpool:
            # main-loop constants first so group 0 can start ASAP; wvt is
            # loaded m-tile 0 first so the very first matmul isn't gated on
            # the full 8KB weight load.
            # HAM warmup: the PE idles ~9us waiting for the first DMAs;
            # ~5us of dummy matmuls flips the clock gate to 2.4 GHz so the
            # real stream starts warm. Pool closes before the main PSUM
            # pools open, freeing the bank.
            with (
                tc.tile_pool(name="warmps", bufs=1, space="PSUM") as warm_pool,
                tc.tile_pool(name="warmsb", bufs=1) as wsb_pool,
            ):
                wdata = wsb_pool.tile([P, 512], bf)
                nc.vector.memset(wdata[:, :], 0.0)
                wps = warm_pool.tile([P, 512], f32)
                for _ in range(12):
                    nc.tensor.matmul(wps[:, :], wdata[:, 0:P], wdata[:, :],
                                     start=True, stop=True,
                                     skip_group_check=True)
            wvt_sb = cpool.tile([P, 4096], bf)
            nc.sync.dma_start(out=wvt_sb[:, 0:1024], in_=wvt_d[:, 0:1024])
            vc0_sb = cpool.tile([P, 4], f32)
            nc.sync.dma_start(out=vc0_sb[:, :], in_=vc0_d[:, :])
            outcat0_sb = cpool.tile([P, 4, Bc // 2], f32)
            outcat1_sb = cpool.tile([P, 4, Bc // 2], f32)

            with (
                tc.tile_pool(name="gd", bufs=3) as gd_pool,
                tc.tile_pool(name="vsb", bufs=2) as v_pool,
                tc.tile_pool(name="junk", bufs=4) as junk_pool,
                tc.tile_pool(name="ps_v", bufs=3, space="PSUM") as pv_pool,
                tc.tile_pool(name="ps_wo", bufs=2, space="PSUM") as wo_pool,
                tc.tile_pool(name="ps_st", bufs=1, space="PSUM") as st_pool,
                tc.tile_pool(name="ps_t", bufs=1, space="PSUM") as pt_pool,
                tc.tile_pool(name="tail", bufs=2) as tail_pool,
            ):
                # prefetch first groups before the tail-only constants;
                # gd0 split so the k=0..3 matmuls gate only on its first half
                gts = []
                gt = gd_pool.tile([P, GD_COLS], bf, tag="gd", name="gt0")
                nc.sync.dma_start(out=gt[:, 0:1568], in_=gd_d[0, :, 0:1568])
                nc.sync.dma_start(out=gt[:, 1568:GD_COLS], in_=gd_d[0, :, 1568:GD_COLS])
                gts.append(gt)
                nc.sync.dma_start(out=wvt_sb[:, 1024:4096], in_=wvt_d[:, 1024:4096])
                for g in range(1, 3):
                    gt = gd_pool.tile([P, GD_COLS], bf, tag="gd", name=f"gt{g}")
                    nc.sync.dma_start(out=gt[:, :], in_=gd_d[g])
                    gts.append(gt)
                # tail-only constants (needed ~100us in; queued behind prefetch)
                wot_sb = cpool.tile([P, 4096], bf)
                nc.sync.dma_start(out=wot_sb[:, :], in_=wot_d[:, :])
                qT_sb = cpool.tile([P, 8 * Bc], f32)
                nc.sync.dma_start(out=qT_sb[:, :], in_=qT_d[:, :])
                grep_sb = cpool.tile([Bc // 2, C], f32)
                nc.sync.dma_start(out=grep_sb[:, :], in_=grep_d[:, :])
                brep_sb = cpool.tile([Bc // 2, C], f32)
                nc.sync.dma_start(out=brep_sb[:, :], in_=brep_d[:, :])
                gdiag_sb = cpool.tile([P, C], f32)
                nc.sync.dma_start(out=gdiag_sb[:, :], in_=gdiag_d[:, :])
                ones_sb = cpool.tile([P, 2], f32)
                nc.vector.memset(ones_sb[:, :], 1.0)

                eps_sb = cpool.tile([Bc // 2, 1], f32)
                nc.vector.memset(eps_sb[:, :], EPS)
                Hn = Bc // 2

                def emit_tail_half(h):
                    # full tail (wo proj, LN, out) for batches h*Hn..(h+1)*Hn
                    occ_h = tail_pool.tile([P, 4, Hn], bf, tag="occ",
                                           name=f"occ{h}")
                    nc.vector.tensor_copy(occ_h[:, :, :],
                                          (outcat0_sb if h == 0 else outcat1_sb)[:, :, :])
                    res_h = tail_pool.tile([P, 8 * Hn], f32, tag="res",
                                           name=f"res{h}")
                    r2_h = tail_pool.tile([P, 8 * Hn], f32, tag="r2",
                                          name=f"r2{h}")
                    stat0 = st_pool.tile([Hn, 2], f32, tag="stat0",
                                         name=f"stat0{h}")
                    stat1 = st_pool.tile([Hn, 2], f32, tag="stat1",
                                         name=f"stat1{h}")
                    for m8 in range(8):
                        ps_wo = wo_pool.tile([P, Hn], f32, tag="ps_wo")
                        for k4 in range(4):
                            nc.tensor.matmul(
                                ps_wo[:, :],
                                wot_sb[:, (m8 * 4 + k4) * P:(m8 * 4 + k4 + 1) * P],
                                occ_h[:, k4, :],
                                start=(k4 == 0), stop=(k4 == 3))
                        r_m = res_h[:, m8 * Hn:(m8 + 1) * Hn]
                        nc.vector.tensor_add(
                            r_m, ps_wo[:, :],
                            qT_sb[:, m8 * Bc + h * Hn:m8 * Bc + (h + 1) * Hn])
                        nc.scalar.square(r2_h[:, m8 * Hn:(m8 + 1) * Hn], r_m)
                    for m8 in range(8):
                        nc.tensor.matmul(stat0[:, :],
                                         res_h[:, m8 * Hn:(m8 + 1) * Hn],
                                         ones_sb[:, :],
                                         start=(m8 == 0), stop=(m8 == 7),
                                         skip_group_check=True)
                        nc.tensor.matmul(stat1[:, :],
                                         r2_h[:, m8 * Hn:(m8 + 1) * Hn],
                                         ones_sb[:, :],
                                         start=(m8 == 0), stop=(m8 == 7),
                                         skip_group_check=True)
                    mean_sb = tail_pool.tile([Hn, 1], f32, tag="mean",
                                             name=f"mean{h}")
                    nc.vector.tensor_scalar(out=mean_sb[:, :], in0=stat0[:, 0:1],
                                            scalar1=1.0 / C, scalar2=None,
                                            op0=Op.mult)
                    ex2_sb = tail_pool.tile([Hn, 1], f32, tag="ex2",
                                            name=f"ex2{h}")
                    nc.vector.tensor_scalar(out=ex2_sb[:, :], in0=stat1[:, 0:1],
                                            scalar1=1.0 / C, scalar2=None,
                                            op0=Op.mult)
                    var_sb = tail_pool.tile([Hn, 1], f32, tag="var",
                                            name=f"var{h}")
                    # var = ex2 - mean^2: (mean*mean - ex2) * -1
                    nc.vector.scalar_tensor_tensor(
                        out=var_sb[:, :], in0=mean_sb[:, :],
                        scalar=mean_sb[:, 0:1], in1=ex2_sb[:, :],
                        op0=Op.mult, op1=Op.subtract)
                    nc.vector.tensor_scalar(out=var_sb[:, :], in0=var_sb[:, :],
                                            scalar1=-1.0, scalar2=None,
                                            op0=Op.mult)
                    sd_sb = tail_pool.tile([Hn, 1], f32, tag="sd",
                                           name=f"sd{h}")
                    nc.scalar.activation(sd_sb[:, :], var_sb[:, :], AF.Sqrt,
                                         bias=eps_sb[:, 0:1])
                    rstd_sb = tail_pool.tile([Hn, 1], f32, tag="rstd",
                                             name=f"rstd{h}")
                    nc.vector.reciprocal(rstd_sb[:, :], sd_sb[:, :])
                    mrneg_sb = tail_pool.tile([Hn, 1], f32, tag="mrneg",
                                              name=f"mrneg{h}")
                    nc.vector.tensor_scalar(out=mrneg_sb[:, :], in0=mean_sb[:, :],
                                            scalar1=rstd_sb[:, 0:1], scalar2=-1.0,
                                            op0=Op.mult, op1=Op.mult)
                    b2_sb = tail_pool.tile([Hn, C], f32, tag="b2",
                                           name=f"b2{h}")
                    nc.vector.scalar_tensor_tensor(
                        out=b2_sb[:, :], in0=grep_sb[:, :],
                        scalar=mrneg_sb[:, 0:1], in1=brep_sb[:, :],
                        op0=Op.mult, op1=Op.add)
                    fin_sb = tail_pool.tile([Hn, C], f32, tag="fin",
                                            name=f"fin{h}")
                    for h2 in range(2):
                        cs = slice(h2 * 512, (h2 + 1) * 512)
                        # scaled transpose: st[b, c] = res[c, b] * g[c]
                        ps_t = pt_pool.tile([Hn, 512], f32, tag="ps_t",
                                            name=f"pst{h}{h2}")
                        for m4 in range(4):
                            m8 = 4 * h2 + m4
                            nc.tensor.matmul(
                                ps_t[:, m4 * P:(m4 + 1) * P],
                                res_h[:, m8 * Hn:(m8 + 1) * Hn],
                                gdiag_sb[:, m8 * P:(m8 + 1) * P],
                                start=True, stop=True, skip_group_check=True)
                        nc.vector.scalar_tensor_tensor(
                            out=fin_sb[:, cs], in0=ps_t[:, :],
                            scalar=rstd_sb[:, 0:1], in1=b2_sb[:, cs],
                            op0=Op.mult, op1=Op.add)
                        nc.sync.dma_start(out=out_d[h * Hn:(h + 1) * Hn, cs],
                                          in_=fin_sb[:, cs])

                for g in range(G):
                    if g < 3:
                        gt = gts[g]
                    else:
                        gt = gd_pool.tile([P, GD_COLS], bf, tag="gd")
                        nc.sync.dma_start(out=gt[:, :], in_=gd_d[g])

                    for m in range(4):
                        ps_v = pv_pool.tile([P, 2 * SP], f32, tag="ps_v")
                        for k in range(8):
                            nc.tensor.matmul(
                                ps_v[:, :],
                                wvt_sb[:, (m * 8 + k) * P:(m * 8 + k + 1) * P],
                                gt[:, k * 2 * SP:(k + 1) * 2 * SP],
                                start=(k == 0), stop=(k == 7))
                        v_sb = v_pool.tile([P, 2, NT], bf, tag=f"v{m}")
                        for j in range(2):
                            nc.scalar.copy(v_sb[:, j, 0:SP],
                                           ps_v[:, j * SP:(j + 1) * SP])
                        if g < 2:
                            # vc0 column persists in this rotation slot
                            for j in range(2):
                                nc.scalar.copy(v_sb[:, j, SP:SP + 1],
                                               vc0_sb[:, m:m + 1])
                        if debug and g == 0 and m == 0:
                            dbg_v_sb = cpool.tile([P, 2 * NT], f32)
                            for j in range(2):
                                nc.vector.tensor_copy(
                                    dbg_v_sb[:, j * NT:(j + 1) * NT], v_sb[:, j, :])
                            nc.sync.dma_start(out=dbg_v_d[:, :], in_=dbg_v_sb[:, :])
                        for j in range(2):
                            junk = junk_pool.tile([P, NT], bf, tag="junk")
                            nc.vector.scalar_tensor_tensor(
                                out=junk[:, :],
                                in0=v_sb[:, j, :],
                                scalar=1.0,
                                in1=gt[:, WC_OFF + m * 2 * NT + j * NT:
                                       WC_OFF + m * 2 * NT + (j + 1) * NT],
                                op0=Op.mult, op1=Op.mult,
                                accum_out=(outcat0_sb if g < 8 else outcat1_sb)[
                                    :, m, 2 * (g % 8) + j:2 * (g % 8) + j + 1])
                    if g == 9:
                        emit_tail_half(0)
                emit_tail_half(1)


    nc.compile()
    return nc


# ---------------------------------------------------------------- entry

def kernel(**inputs):
    from concourse.bass_utils import run_bass_kernel_spmd

    per_core = _host_prep(inputs)
    B = inputs['feature_map'].shape[0]
    assert B == B_FULL, B
    bc = B // NCORES
    in_maps = [per_core(c * bc, (c + 1) * bc) for c in range(NCORES)]

    nc = build_bass(G=bc // 2)
    res = run_bass_kernel_spmd(nc, in_maps, core_ids=list(range(NCORES)))
    out = np.concatenate([r['out'] for r in res.results], axis=0)
    return out.astype(np.float32)


# revision 24
# speedup vs baseline: 1.2687x; 1.2687x over previous
"""Trainium2 Bass kernel for nn_MultiHeadMapAttentionV2.

Math restructuring (v2 — host-side query path extended through softmax):
  - The 5-stage 1x1 conv chain is affine; only the mean token of its output
    feeds the (single) query: queries = W_tot @ mean_spatial(loss_map) + const.
  - pos_kv is pre-added into the feature-map tokens on host (xhat), so the
    device V projection is a single wv @ xhat matmul chain; the mean token
    collapses into the softmax-weight vector (W~_t = p_t + p_0/196) plus a
    constant column vc0 = wv @ (pos_0 - mean(pos_1:)) scaled by p_0.
  - Scores (q-side) are 1.5% of total FLOPs and depend on host-known q, so
    the softmax weights W~ are computed on host and shipped per group
    (197 bf16 cols), removing the on-device scores matmuls + softmax chain.
  - All device matmul operands are bf16 (fp32r streams ~2 cyc/row on HW;
    bf16 streams 1 cyc/row and halves DMA bytes).

Device structure (per core, Bc = 32 batches, G = 16 groups of 2):
  Channels-on-partitions, tokens-on-free. Per group ONE packed bf16 DMA
  ([128, 3336]: 8 fm k-chunks | wc block). Per m-tile: 8 bf16 matmuls
  (wv stationary) -> ps_v [128, 392]; Act copies cast to v_sb bf16
  (third col = vc0, written once per rotation slot); selector matmul
  broadcasts wc [8, 197] -> ps_w [128, 197]; one DVE scalar_tensor_tensor
  per (m, j) does the attention-weighted sum straight into outcat.
  Tail: wo projection on PE, LN stats via ones-matmul, PE transpose to
  [Bc, 1024], normalize.
"""

import numpy as np

P = 128
C = 1024
S = 14
SP = S * S          # 196 spatial tokens
NT = SP + 1         # 197 tokens
H = 8
DK = 64
NCORES = 8
B_FULL = 256
EPS = 1e-5

FM_COLS = 8 * 2 * SP            # 3136 (8 k-chunks x 2 batches x 196)
WC_OFF = FM_COLS                # 3136
WC_COLS = 4 * 2 * NT            # 1576: per m-tile [2*NT] pre-broadcast weights
GD_COLS = WC_OFF + WC_COLS      # 4712


# ---------------------------------------------------------------- host prep

def _host_prep(inputs):
    f = {k: np.ascontiguousarray(np.asarray(v, dtype=np.float32)) for k, v in inputs.items()}
    w1, w2, w3, w4, w5 = f['w1'], f['w2'], f['w3'], f['w4'], f['w5']
    b1, b2, b3, b4, b5 = f['b1'], f['b2'], f['b3'], f['b4'], f['b5']
    B = f['feature_map'].shape[0]

    Wt = w5 @ w4 @ w3 @ w2 @ w1                                   # (1024, 8)
    bt = w5 @ (w4 @ (w3 @ (w2 @ b1 + b2) + b3) + b4) + b5         # (1024,)
    lmean = f['loss_map'].reshape(B, 8, SP).mean(-1)              # (B, 8)
    queries = lmean @ Wt.T + bt + f['pos_q'][0]                   # (B, 1024)
    q = (queries @ f['wq'].T + f['bq']) / np.float32(np.sqrt(DK)) # (B, 512)
    qr_ = q.reshape(B, H, DK)
    wk_r = f['wk'].reshape(H, DK, C)
    Qt = np.einsum('hdc,bhd->bch', wk_r, qr_)                     # (B, 1024, 8)

    pos = f['pos_kv']                                             # (197, 1024)
    c0 = pos[0] - pos[1:].mean(0)                                 # (1024,)
    posT = np.ascontiguousarray(pos[1:].T)                        # (1024, 196)

    fm = f['feature_map'].reshape(B, C, SP)                       # (B, 1024, 196)
    xhat = fm + posT[None]                                        # (B, 1024, 196)

    # ---- host scores + softmax (q-side: ~1.5% of FLOPs)
    # s~_t = Qt . xhat_t ; bk shifts all scores equally -> softmax-invariant
    s_all = np.matmul(Qt.transpose(0, 2, 1), xhat)                # (B, 8, 196)
    sc0 = np.einsum('bch,c->bh', Qt, c0)                          # (B, 8)
    smean = s_all.mean(-1) + sc0                                  # (B, 8) mean-token score
    M = np.maximum(s_all.max(-1), smean)
    p_sp = np.exp(s_all - M[..., None])                           # (B, 8, 196)
    p_m = np.exp(smean - M)                                       # (B, 8)
    den = p_sp.sum(-1) + p_m
    Wsp = (p_sp + p_m[..., None] / SP) / den[..., None]           # (B, 8, 196)
    w0 = p_m / den                                                # (B, 8)

    # ---- shared (batch-independent) device arrays
    import ml_dtypes
    bf16 = ml_dtypes.bfloat16
    wv = f['wv']                                                  # (512, 1024)
    # wvt[p, (m*8+k)*128 + j] = wv[128m+j, 128k+p]
    wvt = np.ascontiguousarray(
        wv.reshape(4, P, 8, P).transpose(3, 0, 2, 1).reshape(P, 4096)).astype(bf16)
    wo = f['wo']                                                  # (1024, 512)
    # wot[p, (m8*4+k4)*128 + j] = wo[128*m8+j, 128*k4+p]
    wot = np.ascontiguousarray(
        wo.reshape(8, P, 4, P).transpose(3, 0, 2, 1).reshape(P, 4096)).astype(bf16)
    # vc0 column per m-tile: vc0a[p, m] = (wv @ c0)[128m + p]
    vc0 = wv @ c0                                                 # (512,)
    vc0a = np.ascontiguousarray(vc0.reshape(4, P).T)              # (128, 4)
    # gdiag[p, m8*128 + c] = ln_g[m8*128 + c] if p == c else 0 — the tail
    # transpose matmul res.T @ gdiag folds the LN gain for free
    gdiag = np.zeros((P, C), np.float32)
    for m8 in range(8):
        gdiag[np.arange(P), m8 * P + np.arange(P)] = f['ln_g'][m8 * P:(m8 + 1) * P]

    shared = {'wvt': wvt, 'wot': wot, 'vc0': vc0a, 'gdiag': gdiag}

    # out bias: v bias bv contributes wo @ bv (sum of p = 1)
    qpb = queries + f['bo'] + f['wo'] @ f['bv']                   # (B, 1024)

    def per_core(bs, be):
        Bc = be - bs
        G = Bc // 2
        gd = np.zeros((G, P, GD_COLS), bf16)
        # fm block: [p, (k*2 + j)*196 + t] = xhat[bs + 2g+j, 128k+p, t]
        gd[:, :, 0:FM_COLS] = (
            xhat[bs:be].reshape(G, 2, 8, P, SP).transpose(0, 3, 2, 1, 4)
            .reshape(G, P, FM_COLS).astype(bf16))
        # pre-broadcast weight block: wcb[g, p, m, j*197 + t] =
        # W~[batch 2g+j, head 2m + p//64, t]; col t=196 is p0 (vc0 weight).
        Wfull = np.concatenate([Wsp, w0[..., None]], -1)          # (B, 8, 197)
        heads = 2 * np.arange(4)[:, None] + (np.arange(P) // 64)[None, :]  # (4,128)
        b0 = Wfull[bs:be:2][:, heads, :]                          # (G, 4, 128, 197)
        b1 = Wfull[bs + 1:be:2][:, heads, :]
        wcb = np.stack([b0, b1], axis=3)                          # (G, 4, 128, 2, 197)
        wcb = wcb.transpose(0, 2, 1, 3, 4).reshape(G, P, WC_COLS)
        gd[:, :, WC_OFF:WC_OFF + WC_COLS] = wcb.astype(bf16)
        # qT[p, m*Bc + b] = qpb[bs + b, 128m + p]
        qT = np.ascontiguousarray(
            qpb[bs:be].T.reshape(8, P, Bc).transpose(1, 0, 2).reshape(P, 8 * Bc))
        grep = np.ascontiguousarray(np.broadcast_to(f['ln_g'], (Bc, C)))
        brep = np.ascontiguousarray(np.broadcast_to(f['ln_b'], (Bc, C)))
        return {'gdata': np.ascontiguousarray(gd), 'qT': qT,
                'grep': grep, 'brep': brep, **shared}

    return per_core


# ---------------------------------------------------------------- bass build

def build_bass(G=16, debug=False):
    import concourse.bacc as bacc
    import concourse.mybir as mybir
    import concourse.tile as tile

    f32 = mybir.dt.float32
    bf = mybir.dt.bfloat16
    Ax = mybir.AxisListType
    Op = mybir.AluOpType
    AF = mybir.ActivationFunctionType

    Bc = 2 * G
    nc = bacc.Bacc(trn_type="TRN2", name="mhma_v2")

    gd_d = nc.dram_tensor('gdata', (G, P, GD_COLS), bf, kind="ExternalInput")
    wvt_d = nc.dram_tensor('wvt', (P, 4096), bf, kind="ExternalInput")
    wot_d = nc.dram_tensor('wot', (P, 4096), bf, kind="ExternalInput")
    vc0_d = nc.dram_tensor('vc0', (P, 4), f32, kind="ExternalInput")
    qT_d = nc.dram_tensor('qT', (P, 8 * Bc), f32, kind="ExternalInput")
    grep_d = nc.dram_tensor('grep', (Bc, C), f32, kind="ExternalInput")
    brep_d = nc.dram_tensor('brep', (Bc, C), f32, kind="ExternalInput")
    gdiag_d = nc.dram_tensor('gdiag', (P, C), f32, kind="ExternalInput")
    out_d = nc.dram_tensor('out', (Bc, C), f32, kind="ExternalOutput")
    if debug:
        dbg_v_d = nc.dram_tensor('dbg_v', (P, 2 * NT), f32, kind="ExternalOutput")
        dbg_oc_d = nc.dram_tensor('dbg_oc', (P, 4 * Bc), f32, kind="ExternalOutput")

    with tile.TileContext(nc) as tc:
        with tc.tile_pool(name="const", bufs=1) as cpool:
            # main-loop constants first so group 0 can start ASAP; wvt is
            # loaded m-tile 0 first so the very first matmul isn't gated on
            # the full 8KB weight load.
            wvt_sb = cpool.tile([P, 4096], bf)
            nc.sync.dma_start(out=wvt_sb[:, 0:1024], in_=wvt_d[:, 0:1024])
            vc0_sb = cpool.tile([P, 4], f32)
            nc.sync.dma_start(out=vc0_sb[:, :], in_=vc0_d[:, :])
            outcat_sb = cpool.tile([P, 4 * Bc], f32)

            with (
                tc.tile_pool(name="gd", bufs=3) as gd_pool,
                tc.tile_pool(name="vsb", bufs=2) as v_pool,
                tc.tile_pool(name="junk", bufs=4) as junk_pool,
                tc.tile_pool(name="ps_v", bufs=4, space="PSUM") as pv_pool,
            ):
                # prefetch first groups before the tail-only constants;
                # gd0 split so the k=0..3 matmuls gate only on its first half
                gts = []
                gt = gd_pool.tile([P, GD_COLS], bf, tag="gd", name="gt0")
                nc.sync.dma_start(out=gt[:, 0:1568], in_=gd_d[0, :, 0:1568])
                nc.sync.dma_start(out=gt[:, 1568:GD_COLS], in_=gd_d[0, :, 1568:GD_COLS])
                gts.append(gt)
                nc.sync.dma_start(out=wvt_sb[:, 1024:4096], in_=wvt_d[:, 1024:4096])
                for g in range(1, 3):
                    gt = gd_pool.tile([P, GD_COLS], bf, tag="gd", name=f"gt{g}")
                    nc.sync.dma_start(out=gt[:, :], in_=gd_d[g])
                    gts.append(gt)
                # tail-only constants (needed ~100us in; queued behind prefetch)
                wot_sb = cpool.tile([P, 4096], bf)
                nc.sync.dma_start(out=wot_sb[:, :], in_=wot_d[:, :])
                qT_sb = cpool.tile([P, 8 * Bc], f32)
                nc.sync.dma_start(out=qT_sb[:, :], in_=qT_d[:, :])
                grep_sb = cpool.tile([Bc, C], f32)
                nc.sync.dma_start(out=grep_sb[:, :], in_=grep_d[:, :])
                brep_sb = cpool.tile([Bc, C], f32)
                nc.sync.dma_start(out=brep_sb[:, :], in_=brep_d[:, :])
                gdiag_sb = cpool.tile([P, C], f32)
                nc.sync.dma_start(out=gdiag_sb[:, :], in_=gdiag_d[:, :])
                ones_sb = cpool.tile([P, 2], f32)
                nc.vector.memset(ones_sb[:, :], 1.0)

                for g in range(G):
                    if g < 3:
                        gt = gts[g]
                    else:
                        gt = gd_pool.tile([P, GD_COLS], bf, tag="gd")
                        nc.sync.dma_start(out=gt[:, :], in_=gd_d[g])

                    for m in range(4):
                        ps_v = pv_pool.tile([P, 2 * SP], f32, tag="ps_v")
                        for k in range(8):
                            nc.tensor.matmul(
                                ps_v[:, :],
                                wvt_sb[:, (m * 8 + k) * P:(m * 8 + k + 1) * P],
                                gt[:, k * 2 * SP:(k + 1) * 2 * SP],
                                start=(k == 0), stop=(k == 7))
                        v_sb = v_pool.tile([P, 2, NT], bf, tag=f"v{m}")
                        for j in range(2):
                            nc.scalar.copy(v_sb[:, j, 0:SP],
                                           ps_v[:, j * SP:(j + 1) * SP])
                        if g < 2:
                            # vc0 column persists in this rotation slot
                            for j in range(2):
                                nc.scalar.copy(v_sb[:, j, SP:SP + 1],
                                               vc0_sb[:, m:m + 1])
                        for j in range(2):
                            junk = junk_pool.tile([P, NT], bf, tag="junk")
                            nc.vector.scalar_tensor_tensor(
                                out=junk[:, :],
                                in0=v_sb[:, j, :],
                                scalar=1.0,
                                in1=gt[:, WC_OFF + m * 2 * NT + j * NT:
                                       WC_OFF + m * 2 * NT + (j + 1) * NT],
                                op0=Op.mult, op1=Op.mult,
                                accum_out=outcat_sb[:, m * Bc + 2 * g + j:
                                                    m * Bc + 2 * g + j + 1])

            # ---- tail: wo projection, LN stats, scaled transpose, normalize
            with (
                tc.tile_pool(name="ps_wo", bufs=2, space="PSUM") as wo_pool,
                tc.tile_pool(name="ps_st", bufs=1, space="PSUM") as st_pool,
                tc.tile_pool(name="ps_t", bufs=1, space="PSUM") as pt_pool,
                tc.tile_pool(name="tail", bufs=1) as tail_pool,
            ):
                # bf16 copy of outcat so the wo matmuls run all-bf16 (FWL)
                occ_sb = tail_pool.tile([P, 4 * Bc], bf)
                nc.vector.tensor_copy(occ_sb[:, :], outcat_sb[:, :])
                res_sb = tail_pool.tile([P, 8 * Bc], f32)
                r2_sb = tail_pool.tile([P, 8 * Bc], f32)
                stat0 = st_pool.tile([Bc, 2], f32)
                stat1 = st_pool.tile([Bc, 2], f32)
                # phase 1: all wo projections; DVE adds + Act squares trail
                for m8 in range(8):
                    ps_wo = wo_pool.tile([P, Bc], f32, tag="ps_wo")
                    for k4 in range(4):
                        nc.tensor.matmul(
                            ps_wo[:, :],
                            wot_sb[:, (m8 * 4 + k4) * P:(m8 * 4 + k4 + 1) * P],
                            occ_sb[:, k4 * Bc:(k4 + 1) * Bc],
                            start=(k4 == 0), stop=(k4 == 3))
                    r_m = res_sb[:, m8 * Bc:(m8 + 1) * Bc]
                    nc.vector.tensor_add(r_m, ps_wo[:, :], qT_sb[:, m8 * Bc:(m8 + 1) * Bc])
                    nc.scalar.square(r2_sb[:, m8 * Bc:(m8 + 1) * Bc], r_m)
                # phase 2: stats matmuls back-to-back
                for m8 in range(8):
                    nc.tensor.matmul(stat0[:, :], res_sb[:, m8 * Bc:(m8 + 1) * Bc],
                                     ones_sb[:, :],
                                     start=(m8 == 0), stop=(m8 == 7),
                                     skip_group_check=True)
                    nc.tensor.matmul(stat1[:, :], r2_sb[:, m8 * Bc:(m8 + 1) * Bc],
                                     ones_sb[:, :],
                                     start=(m8 == 0), stop=(m8 == 7),
                                     skip_group_check=True)
                mean_sb = tail_pool.tile([Bc, 1], f32)
                nc.vector.tensor_scalar(out=mean_sb[:, :], in0=stat0[:, 0:1],
                                        scalar1=1.0 / C, scalar2=None, op0=Op.mult)
                ex2_sb = tail_pool.tile([Bc, 1], f32)
                nc.vector.tensor_scalar(out=ex2_sb[:, :], in0=stat1[:, 0:1],
                                        scalar1=1.0 / C, scalar2=None, op0=Op.mult)
                var_sb = tail_pool.tile([Bc, 1], f32)
                # var = ex2 - mean^2: (mean*mean - ex2) * -1
                nc.vector.scalar_tensor_tensor(
                    out=var_sb[:, :], in0=mean_sb[:, :], scalar=mean_sb[:, 0:1],
                    in1=ex2_sb[:, :], op0=Op.mult, op1=Op.subtract)
                nc.vector.tensor_scalar(out=var_sb[:, :], in0=var_sb[:, :],
                                        scalar1=-1.0, scalar2=None, op0=Op.mult)
                eps_sb = tail_pool.tile([Bc, 1], f32)
                nc.vector.memset(eps_sb[:, :], EPS)
                sd_sb = tail_pool.tile([Bc, 1], f32)
                nc.scalar.activation(sd_sb[:, :], var_sb[:, :], AF.Sqrt,
                                     bias=eps_sb[:, 0:1])
                rstd_sb = tail_pool.tile([Bc, 1], f32)
                nc.vector.reciprocal(rstd_sb[:, :], sd_sb[:, :])
                # scaled transpose: st[b, c] = res[c, b] * g[c]
                ps_t = pt_pool.tile([Bc, C], f32)
                for m8 in range(8):
                    nc.tensor.matmul(
                        ps_t[:, m8 * P:(m8 + 1) * P],
                        res_sb[:, m8 * Bc:(m8 + 1) * Bc],
                        gdiag_sb[:, m8 * P:(m8 + 1) * P],
                        start=True, stop=True, skip_group_check=True)
                # B2 = brep - mean*rstd*grep  (per-batch scalar via stt)
                mrneg_sb = tail_pool.tile([Bc, 1], f32)
                nc.vector.tensor_scalar(out=mrneg_sb[:, :], in0=mean_sb[:, :],
                                        scalar1=rstd_sb[:, 0:1], scalar2=-1.0,
                                        op0=Op.mult, op1=Op.mult)
                b2_sb = tail_pool.tile([Bc, C], f32)
                nc.vector.scalar_tensor_tensor(
                    out=b2_sb[:, :], in0=grep_sb[:, :], scalar=mrneg_sb[:, 0:1],
                    in1=brep_sb[:, :], op0=Op.mult, op1=Op.add)
                # fin = st*rstd + B2, in halves so the first DMA overlaps
                fin_sb = tail_pool.tile([Bc, C], f32)
                for h2 in range(2):
                    cs = slice(h2 * 512, (h2 + 1) * 512)
                    nc.vector.scalar_tensor_tensor(
                        out=fin_sb[:, cs], in0=ps_t[:, cs],
                        scalar=rstd_sb[:, 0:1], in1=b2_sb[:, cs],
                        op0=Op.mult, op1=Op.add)
                    nc.sync.dma_start(out=out_d[:, cs], in_=fin_sb[:, cs])

    nc.compile()
    return nc


# ---------------------------------------------------------------- entry

def kernel(**inputs):
    from concourse.bass_utils import run_bass_kernel_spmd

    per_core = _host_prep(inputs)
    B = inputs['feature_map'].shape[0]
    assert B == B_FULL, B
    bc = B // NCORES
    in_maps = [per_core(c * bc, (c + 1) * bc) for c in range(NCORES)]

    nc = build_bass(G=bc // 2)
    res = run_bass_kernel_spmd(nc, in_maps, core_ids=list(range(NCORES)))
    out = np.concatenate([r['out'] for r in res.results], axis=0)
    return out.astype(np.float32)


# revision 25
# speedup vs baseline: 1.2755x; 1.0053x over previous
"""Trainium2 Bass kernel for nn_MultiHeadMapAttentionV2.

Math restructuring (v2 — host-side query path extended through softmax):
  - The 5-stage 1x1 conv chain is affine; only the mean token of its output
    feeds the (single) query: queries = W_tot @ mean_spatial(loss_map) + const.
  - pos_kv is pre-added into the feature-map tokens on host (xhat), so the
    device V projection is a single wv @ xhat matmul chain; the mean token
    collapses into the softmax-weight vector (W~_t = p_t + p_0/196) plus a
    constant column vc0 = wv @ (pos_0 - mean(pos_1:)) scaled by p_0.
  - Scores (q-side) are 1.5% of total FLOPs and depend on host-known q, so
    the softmax weights W~ are computed on host and shipped per group
    (197 bf16 cols), removing the on-device scores matmuls + softmax chain.
  - All device matmul operands are bf16 (fp32r streams ~2 cyc/row on HW;
    bf16 streams 1 cyc/row and halves DMA bytes).

Device structure (per core, Bc = 32 batches, G = 16 groups of 2):
  Channels-on-partitions, tokens-on-free. Per group ONE packed bf16 DMA
  ([128, 3336]: 8 fm k-chunks | wc block). Per m-tile: 8 bf16 matmuls
  (wv stationary) -> ps_v [128, 392]; Act copies cast to v_sb bf16
  (third col = vc0, written once per rotation slot); selector matmul
  broadcasts wc [8, 197] -> ps_w [128, 197]; one DVE scalar_tensor_tensor
  per (m, j) does the attention-weighted sum straight into outcat.
  Tail: wo projection on PE, LN stats via ones-matmul, PE transpose to
  [Bc, 1024], normalize.
"""

import numpy as np

P = 128
C = 1024
S = 14
SP = S * S          # 196 spatial tokens
NT = SP + 1         # 197 tokens
H = 8
DK = 64
NCORES = 8
B_FULL = 256
EPS = 1e-5

FM_COLS = 8 * 2 * SP            # 3136 (8 k-chunks x 2 batches x 196)
WC_OFF = FM_COLS                # 3136
WC_COLS = 4 * 2 * NT            # 1576: per m-tile [2*NT] pre-broadcast weights
GD_COLS = WC_OFF + WC_COLS      # 4712


# ---------------------------------------------------------------- host prep

def _host_prep(inputs):
    f = {k: np.ascontiguousarray(np.asarray(v, dtype=np.float32)) for k, v in inputs.items()}
    w1, w2, w3, w4, w5 = f['w1'], f['w2'], f['w3'], f['w4'], f['w5']
    b1, b2, b3, b4, b5 = f['b1'], f['b2'], f['b3'], f['b4'], f['b5']
    B = f['feature_map'].shape[0]

    Wt = w5 @ w4 @ w3 @ w2 @ w1                                   # (1024, 8)
    bt = w5 @ (w4 @ (w3 @ (w2 @ b1 + b2) + b3) + b4) + b5         # (1024,)
    lmean = f['loss_map'].reshape(B, 8, SP).mean(-1)              # (B, 8)
    queries = lmean @ Wt.T + bt + f['pos_q'][0]                   # (B, 1024)
    q = (queries @ f['wq'].T + f['bq']) / np.float32(np.sqrt(DK)) # (B, 512)
    qr_ = q.reshape(B, H, DK)
    wk_r = f['wk'].reshape(H, DK, C)
    Qt = np.einsum('hdc,bhd->bch', wk_r, qr_)                     # (B, 1024, 8)

    pos = f['pos_kv']                                             # (197, 1024)
    c0 = pos[0] - pos[1:].mean(0)                                 # (1024,)
    posT = np.ascontiguousarray(pos[1:].T)                        # (1024, 196)

    fm = f['feature_map'].reshape(B, C, SP)                       # (B, 1024, 196)
    xhat = fm + posT[None]                                        # (B, 1024, 196)

    # ---- host scores + softmax (q-side: ~1.5% of FLOPs)
    # s~_t = Qt . xhat_t ; bk shifts all scores equally -> softmax-invariant
    s_all = np.matmul(Qt.transpose(0, 2, 1), xhat)                # (B, 8, 196)
    sc0 = np.einsum('bch,c->bh', Qt, c0)                          # (B, 8)
    smean = s_all.mean(-1) + sc0                                  # (B, 8) mean-token score
    M = np.maximum(s_all.max(-1), smean)
    p_sp = np.exp(s_all - M[..., None])                           # (B, 8, 196)
    p_m = np.exp(smean - M)                                       # (B, 8)
    den = p_sp.sum(-1) + p_m
    Wsp = (p_sp + p_m[..., None] / SP) / den[..., None]           # (B, 8, 196)
    w0 = p_m / den                                                # (B, 8)

    # ---- shared (batch-independent) device arrays
    import ml_dtypes
    bf16 = ml_dtypes.bfloat16
    wv = f['wv']                                                  # (512, 1024)
    # wvt[p, (m*8+k)*128 + j] = wv[128m+j, 128k+p]
    wvt = np.ascontiguousarray(
        wv.reshape(4, P, 8, P).transpose(3, 0, 2, 1).reshape(P, 4096)).astype(bf16)
    wo = f['wo']                                                  # (1024, 512)
    # wot[p, (m8*4+k4)*128 + j] = wo[128*m8+j, 128*k4+p]
    wot = np.ascontiguousarray(
        wo.reshape(8, P, 4, P).transpose(3, 0, 2, 1).reshape(P, 4096)).astype(bf16)
    # vc0 column per m-tile: vc0a[p, m] = (wv @ c0)[128m + p]
    vc0 = wv @ c0                                                 # (512,)
    vc0a = np.ascontiguousarray(vc0.reshape(4, P).T)              # (128, 4)
    # gdiag[p, m8*128 + c] = ln_g[m8*128 + c] if p == c else 0 — the tail
    # transpose matmul res.T @ gdiag folds the LN gain for free
    gdiag = np.zeros((P, C), np.float32)
    for m8 in range(8):
        gdiag[np.arange(P), m8 * P + np.arange(P)] = f['ln_g'][m8 * P:(m8 + 1) * P]

    shared = {'wvt': wvt, 'wot': wot, 'vc0': vc0a, 'gdiag': gdiag}

    # out bias: v bias bv contributes wo @ bv (sum of p = 1)
    qpb = queries + f['bo'] + f['wo'] @ f['bv']                   # (B, 1024)

    def per_core(bs, be):
        Bc = be - bs
        G = Bc // 2
        gd = np.zeros((G, P, GD_COLS), bf16)
        # fm block: [p, (k*2 + j)*196 + t] = xhat[bs + 2g+j, 128k+p, t]
        gd[:, :, 0:FM_COLS] = (
            xhat[bs:be].reshape(G, 2, 8, P, SP).transpose(0, 3, 2, 1, 4)
            .reshape(G, P, FM_COLS).astype(bf16))
        # pre-broadcast weight block: wcb[g, p, m, j*197 + t] =
        # W~[batch 2g+j, head 2m + p//64, t]; col t=196 is p0 (vc0 weight).
        Wfull = np.concatenate([Wsp, w0[..., None]], -1)          # (B, 8, 197)
        heads = 2 * np.arange(4)[:, None] + (np.arange(P) // 64)[None, :]  # (4,128)
        b0 = Wfull[bs:be:2][:, heads, :]                          # (G, 4, 128, 197)
        b1 = Wfull[bs + 1:be:2][:, heads, :]
        wcb = np.stack([b0, b1], axis=3)                          # (G, 4, 128, 2, 197)
        wcb = wcb.transpose(0, 2, 1, 3, 4).reshape(G, P, WC_COLS)
        gd[:, :, WC_OFF:WC_OFF + WC_COLS] = wcb.astype(bf16)
        # qT[p, m*Bc + b] = qpb[bs + b, 128m + p]
        qT = np.ascontiguousarray(
            qpb[bs:be].T.reshape(8, P, Bc).transpose(1, 0, 2).reshape(P, 8 * Bc))
        grep = np.ascontiguousarray(np.broadcast_to(f['ln_g'], (Bc, C)))
        brep = np.ascontiguousarray(np.broadcast_to(f['ln_b'], (Bc, C)))
        return {'gdata': np.ascontiguousarray(gd), 'qT': qT,
                'grep': grep, 'brep': brep, **shared}

    return per_core


# ---------------------------------------------------------------- bass build

def build_bass(G=16, debug=False):
    import concourse.bacc as bacc
    import concourse.mybir as mybir
    import concourse.tile as tile

    f32 = mybir.dt.float32
    bf = mybir.dt.bfloat16
    Ax = mybir.AxisListType
    Op = mybir.AluOpType
    AF = mybir.ActivationFunctionType

    Bc = 2 * G
    nc = bacc.Bacc(trn_type="TRN2", name="mhma_v2")

    gd_d = nc.dram_tensor('gdata', (G, P, GD_COLS), bf, kind="ExternalInput")
    wvt_d = nc.dram_tensor('wvt', (P, 4096), bf, kind="ExternalInput")
    wot_d = nc.dram_tensor('wot', (P, 4096), bf, kind="ExternalInput")
    vc0_d = nc.dram_tensor('vc0', (P, 4), f32, kind="ExternalInput")
    qT_d = nc.dram_tensor('qT', (P, 8 * Bc), f32, kind="ExternalInput")
    grep_d = nc.dram_tensor('grep', (Bc, C), f32, kind="ExternalInput")
    brep_d = nc.dram_tensor('brep', (Bc, C), f32, kind="ExternalInput")
    gdiag_d = nc.dram_tensor('gdiag', (P, C), f32, kind="ExternalInput")
    out_d = nc.dram_tensor('out', (Bc, C), f32, kind="ExternalOutput")
    if debug:
        dbg_v_d = nc.dram_tensor('dbg_v', (P, 2 * NT), f32, kind="ExternalOutput")
        dbg_oc_d = nc.dram_tensor('dbg_oc', (P, 4 * Bc), f32, kind="ExternalOutput")

    with tile.TileContext(nc) as tc:
        with tc.tile_pool(name="const", bufs=1) as cpool:
            # main-loop constants first so group 0 can start ASAP; wvt is
            # loaded m-tile 0 first so the very first matmul isn't gated on
            # the full 8KB weight load.
            wvt_sb = cpool.tile([P, 4096], bf)
            nc.sync.dma_start(out=wvt_sb[:, 0:1024], in_=wvt_d[:, 0:1024])
            vc0_sb = cpool.tile([P, 4], f32)
            nc.sync.dma_start(out=vc0_sb[:, :], in_=vc0_d[:, :])
            outcat_sb = cpool.tile([P, 4 * Bc], f32)

            with (
                tc.tile_pool(name="gd", bufs=3) as gd_pool,
                tc.tile_pool(name="vsb", bufs=2) as v_pool,
                tc.tile_pool(name="junk", bufs=4) as junk_pool,
                tc.tile_pool(name="ps_v", bufs=5, space="PSUM") as pv_pool,
            ):
                # prefetch first groups before the tail-only constants;
                # gd0 split so the k=0..3 matmuls gate only on its first half
                gts = []
                gt = gd_pool.tile([P, GD_COLS], bf, tag="gd", name="gt0")
                nc.sync.dma_start(out=gt[:, 0:1568], in_=gd_d[0, :, 0:1568])
                nc.sync.dma_start(out=gt[:, 1568:GD_COLS], in_=gd_d[0, :, 1568:GD_COLS])
                gts.append(gt)
                nc.sync.dma_start(out=wvt_sb[:, 1024:4096], in_=wvt_d[:, 1024:4096])
                for g in range(1, 3):
                    gt = gd_pool.tile([P, GD_COLS], bf, tag="gd", name=f"gt{g}")
                    nc.sync.dma_start(out=gt[:, :], in_=gd_d[g])
                    gts.append(gt)
                # tail-only constants: tiles now, DMAs issued after the
                # group-DMA stream so they don't starve groups 3-4
                wot_sb = cpool.tile([P, 4096], bf)
                qT_sb = cpool.tile([P, 8 * Bc], f32)
                grep_sb = cpool.tile([Bc, C], f32)
                brep_sb = cpool.tile([Bc, C], f32)
                gdiag_sb = cpool.tile([P, C], f32)
                ones_sb = cpool.tile([P, 2], f32)
                nc.vector.memset(ones_sb[:, :], 1.0)

                for g in range(G):
                    if g < 3:
                        gt = gts[g]
                    else:
                        gt = gd_pool.tile([P, GD_COLS], bf, tag="gd")
                        nc.sync.dma_start(out=gt[:, :], in_=gd_d[g])

                    for m in range(4):
                        ps_v = pv_pool.tile([P, 2 * SP], f32, tag="ps_v")
                        for k in range(8):
                            nc.tensor.matmul(
                                ps_v[:, :],
                                wvt_sb[:, (m * 8 + k) * P:(m * 8 + k + 1) * P],
                                gt[:, k * 2 * SP:(k + 1) * 2 * SP],
                                start=(k == 0), stop=(k == 7))
                        v_sb = v_pool.tile([P, 2, NT], bf, tag=f"v{m}")
                        for j in range(2):
                            nc.scalar.copy(v_sb[:, j, 0:SP],
                                           ps_v[:, j * SP:(j + 1) * SP])
                        if g < 2:
                            # vc0 column persists in this rotation slot
                            for j in range(2):
                                nc.scalar.copy(v_sb[:, j, SP:SP + 1],
                                               vc0_sb[:, m:m + 1])
                        for j in range(2):
                            junk = junk_pool.tile([P, NT], bf, tag="junk")
                            nc.vector.scalar_tensor_tensor(
                                out=junk[:, :],
                                in0=v_sb[:, j, :],
                                scalar=1.0,
                                in1=gt[:, WC_OFF + m * 2 * NT + j * NT:
                                       WC_OFF + m * 2 * NT + (j + 1) * NT],
                                op0=Op.mult, op1=Op.mult,
                                accum_out=outcat_sb[:, m * Bc + 2 * g + j:
                                                    m * Bc + 2 * g + j + 1])
                    if g == 6:
                        nc.sync.dma_start(out=wot_sb[:, :], in_=wot_d[:, :])
                        nc.sync.dma_start(out=qT_sb[:, :], in_=qT_d[:, :])
                        nc.sync.dma_start(out=grep_sb[:, :], in_=grep_d[:, :])
                        nc.sync.dma_start(out=brep_sb[:, :], in_=brep_d[:, :])
                        nc.sync.dma_start(out=gdiag_sb[:, :], in_=gdiag_d[:, :])

            # ---- tail: wo projection, LN stats, scaled transpose, normalize
            with (
                tc.tile_pool(name="ps_wo", bufs=2, space="PSUM") as wo_pool,
                tc.tile_pool(name="ps_st", bufs=1, space="PSUM") as st_pool,
                tc.tile_pool(name="ps_t", bufs=1, space="PSUM") as pt_pool,
                tc.tile_pool(name="tail", bufs=1) as tail_pool,
            ):
                # bf16 copy of outcat so the wo matmuls run all-bf16 (FWL)
                occ_sb = tail_pool.tile([P, 4 * Bc], bf)
                nc.vector.tensor_copy(occ_sb[:, :], outcat_sb[:, :])
                res_sb = tail_pool.tile([P, 8 * Bc], f32)
                r2_sb = tail_pool.tile([P, 8 * Bc], f32)
                stat0 = st_pool.tile([Bc, 2], f32)
                stat1 = st_pool.tile([Bc, 2], f32)
                # phase 1: all wo projections; DVE adds + Act squares trail
                for m8 in range(8):
                    ps_wo = wo_pool.tile([P, Bc], f32, tag="ps_wo")
                    for k4 in range(4):
                        nc.tensor.matmul(
                            ps_wo[:, :],
                            wot_sb[:, (m8 * 4 + k4) * P:(m8 * 4 + k4 + 1) * P],
                            occ_sb[:, k4 * Bc:(k4 + 1) * Bc],
                            start=(k4 == 0), stop=(k4 == 3))
                    r_m = res_sb[:, m8 * Bc:(m8 + 1) * Bc]
                    nc.vector.tensor_add(r_m, ps_wo[:, :], qT_sb[:, m8 * Bc:(m8 + 1) * Bc])
                    nc.scalar.square(r2_sb[:, m8 * Bc:(m8 + 1) * Bc], r_m)
                # phase 2: stats matmuls back-to-back
                for m8 in range(8):
                    nc.tensor.matmul(stat0[:, :], res_sb[:, m8 * Bc:(m8 + 1) * Bc],
                                     ones_sb[:, :],
                                     start=(m8 == 0), stop=(m8 == 7),
                                     skip_group_check=True)
                    nc.tensor.matmul(stat1[:, :], r2_sb[:, m8 * Bc:(m8 + 1) * Bc],
                                     ones_sb[:, :],
                                     start=(m8 == 0), stop=(m8 == 7),
                                     skip_group_check=True)
                mean_sb = tail_pool.tile([Bc, 1], f32)
                nc.vector.tensor_scalar(out=mean_sb[:, :], in0=stat0[:, 0:1],
                                        scalar1=1.0 / C, scalar2=None, op0=Op.mult)
                ex2_sb = tail_pool.tile([Bc, 1], f32)
                nc.vector.tensor_scalar(out=ex2_sb[:, :], in0=stat1[:, 0:1],
                                        scalar1=1.0 / C, scalar2=None, op0=Op.mult)
                var_sb = tail_pool.tile([Bc, 1], f32)
                # var = ex2 - mean^2: (mean*mean - ex2) * -1
                nc.vector.scalar_tensor_tensor(
                    out=var_sb[:, :], in0=mean_sb[:, :], scalar=mean_sb[:, 0:1],
                    in1=ex2_sb[:, :], op0=Op.mult, op1=Op.subtract)
                nc.vector.tensor_scalar(out=var_sb[:, :], in0=var_sb[:, :],
                                        scalar1=-1.0, scalar2=None, op0=Op.mult)
                eps_sb = tail_pool.tile([Bc, 1], f32)
                nc.vector.memset(eps_sb[:, :], EPS)
                sd_sb = tail_pool.tile([Bc, 1], f32)
                nc.scalar.activation(sd_sb[:, :], var_sb[:, :], AF.Sqrt,
                                     bias=eps_sb[:, 0:1])
                rstd_sb = tail_pool.tile([Bc, 1], f32)
                nc.vector.reciprocal(rstd_sb[:, :], sd_sb[:, :])
                # scaled transpose: st[b, c] = res[c, b] * g[c]
                ps_t = pt_pool.tile([Bc, C], f32)
                for m8 in range(8):
                    nc.tensor.matmul(
                        ps_t[:, m8 * P:(m8 + 1) * P],
                        res_sb[:, m8 * Bc:(m8 + 1) * Bc],
                        gdiag_sb[:, m8 * P:(m8 + 1) * P],
                        start=True, stop=True, skip_group_check=True)
                # B2 = brep - mean*rstd*grep  (per-batch scalar via stt)
                mrneg_sb = tail_pool.tile([Bc, 1], f32)
                nc.vector.tensor_scalar(out=mrneg_sb[:, :], in0=mean_sb[:, :],
                                        scalar1=rstd_sb[:, 0:1], scalar2=-1.0,
                                        op0=Op.mult, op1=Op.mult)
                b2_sb = tail_pool.tile([Bc, C], f32)
                nc.vector.scalar_tensor_tensor(
                    out=b2_sb[:, :], in0=grep_sb[:, :], scalar=mrneg_sb[:, 0:1],
                    in1=brep_sb[:, :], op0=Op.mult, op1=Op.add)
                # fin = st*rstd + B2, in halves so the first DMA overlaps
                fin_sb = tail_pool.tile([Bc, C], f32)
                for h2 in range(2):
                    cs = slice(h2 * 512, (h2 + 1) * 512)
                    nc.vector.scalar_tensor_tensor(
                        out=fin_sb[:, cs], in0=ps_t[:, cs],
                        scalar=rstd_sb[:, 0:1], in1=b2_sb[:, cs],
                        op0=Op.mult, op1=Op.add)
                    nc.sync.dma_start(out=out_d[:, cs], in_=fin_sb[:, cs])

    nc.compile()
    return nc


# ---------------------------------------------------------------- entry

def kernel(**inputs):
    from concourse.bass_utils import run_bass_kernel_spmd

    per_core = _host_prep(inputs)
    B = inputs['feature_map'].shape[0]
    assert B == B_FULL, B
    bc = B // NCORES
    in_maps = [per_core(c * bc, (c + 1) * bc) for c in range(NCORES)]

    nc = build_bass(G=bc // 2)
    res = run_bass_kernel_spmd(nc, in_maps, core_ids=list(range(NCORES)))
    out = np.concatenate([r['out'] for r in res.results], axis=0)
    return out.astype(np.float32)
